# revision 1
# baseline (speedup 1.0000x reference)
import os
import numpy as np

# nn_Encoder_77455440216069 — graph transformer encoder (CiteSeer-like).
# Strategy: run the full model (embed + 7 encoder layers) on the 8
# axon-tunneled trn2 NeuronCores via jax/PJRT, node-sharded (1250 nodes
# per core, dst-partitioned edges, replicated params), with h exchanged
# by all_gather between layers.  Falls back to host NumPy on any failure.
N = 10000      # nodes
E = 160000     # edges
IN = 3703      # input features
D = 256        # d_model
DK = 16        # d_k
DV = 16        # d_v
H = 32         # num_heads
L = 7          # encoder layers
NC = 8         # cores
NLOC = N // NC  # 1250

LAST_HW_EXEC_NS = 0


# ---------------------------------------------------------------- host path
def _layer_norm_np(h, g, b, eps=1e-5):
    m = h.mean(-1, keepdims=True, dtype=np.float32)
    v = ((h - m) ** 2).mean(-1, keepdims=True, dtype=np.float32)
    return (h - m) / np.sqrt(v + eps) * g + b


def _host_kernel(x, src, dst, W_embed, Wq, Wk, Wv, Wo, bo, Wm, bm, g_ln,
                 b_ln, g_mlp, b_mlp):
    order = np.argsort(dst, kind="stable")
    s_s = src[order]
    d_s = dst[order]
    starts = np.concatenate(([0], np.nonzero(np.diff(d_s))[0] + 1))
    seg_dst = d_s[starts]
    h = x @ W_embed.T
    inv = np.float32(1.0 / np.sqrt(np.float32(DK)))
    for l in range(L):
        Q = (h @ Wq[l].T).reshape(N, H, DK).swapaxes(1, 2)
        K = (h @ Wk[l].T).reshape(N, H, DK).swapaxes(1, 2)
        V = (h @ Wv[l].T).reshape(N, H, DV).swapaxes(1, 2)
        alpha = np.matmul(Q[d_s], K[s_s].swapaxes(1, 2)) * inv
        alpha -= alpha.max(-1, keepdims=True)
        np.exp(alpha, out=alpha)
        alpha /= alpha.sum(-1, keepdims=True)
        msg = np.matmul(alpha, V[s_s])
        seg = np.add.reduceat(msg.reshape(len(s_s), DK * H), starts, axis=0)
        agg = np.zeros((N, DK, H), np.float32)
        agg[seg_dst] = seg.reshape(-1, DK, H)
        attn_out = agg.swapaxes(1, 2).reshape(N, H * DV)
        h1 = _layer_norm_np(h + attn_out @ Wo[l].T + bo[l], g_ln[l], b_ln[l])
        h2 = h1 + h1 @ Wm[l].T + bm[l]
        h = _layer_norm_np(h2, g_mlp[l], b_mlp[l])
    return h.astype(np.float32)


# -------------------------------------------------------------- device path
def _prep_edges(src, dst):
    """Sort edges by dst, shard by owning core, pad each shard to equal
    length with edges pointing at a trash row (node NLOC in local ids)."""
    order = np.argsort(dst, kind="stable")
    s_s = src[order].astype(np.int32)
    d_s = dst[order].astype(np.int32)
    shard = d_s // NLOC
    counts = np.bincount(shard, minlength=NC)
    emax = int(counts.max())
    emax = ((emax + 127) // 128) * 128
    src_sh = np.zeros((NC, emax), np.int32)
    dstl_sh = np.full((NC, emax), NLOC, np.int32)   # pad -> trash row
    for c in range(NC):
        sel = shard == c
        k = int(counts[c])
        src_sh[c, :k] = s_s[sel]
        dstl_sh[c, :k] = d_s[sel] - c * NLOC
    return src_sh, dstl_sh


def _device_kernel(x, src, dst, W_embed, Wq, Wk, Wv, Wo, bo, Wm, bm, g_ln,
                   b_ln, g_mlp, b_mlp):
    import jax
    import jax.numpy as jnp
    from jax.sharding import Mesh, PartitionSpec as P
    from jax.experimental.shard_map import shard_map

    devs = jax.devices()
    if len(devs) < NC:
        raise RuntimeError(f"need {NC} devices, have {len(devs)}")
    mesh = Mesh(np.array(devs[:NC]), ("x",))

    src_sh, dstl_sh = _prep_edges(src, dst)

    x_sh = x.reshape(NC, NLOC, IN)

    inv = np.float32(1.0 / np.sqrt(np.float32(DK)))

    def ln(h, g, b, eps=1e-5):
        m = h.mean(-1, keepdims=True)
        v = ((h - m) ** 2).mean(-1, keepdims=True)
        return (h - m) * jax.lax.rsqrt(v + eps) * g + b

    def embed_body(x_l):
        h_loc = x_l[0] @ W_embed.T                  # [NLOC, D]
        return h_loc[None]

    def edge_body(h_l, src_l, dstl_l, Wq, Wk, Wv):
        h_loc = h_l[0]
        src_l = src_l[0]
        dstl_l = dstl_l[0]
        h_full = jax.lax.all_gather(h_loc, "x", tiled=True)  # [N, D]
        K = (h_full @ Wk.T).reshape(N, H, DK)       # [N, h, b]
        V = (h_full @ Wv.T).reshape(N, H, DV)       # [N, h, b]
        Q = (h_loc @ Wq.T).reshape(NLOC, H, DK)     # [NLOC, h, a]
        Qd = Q[dstl_l % NLOC]                       # [ES, h, a]
        Ks = K[src_l]                               # [ES, h, b]
        Vs = V[src_l]
        alpha = jnp.einsum("eha,ehb->eab", Qd, Ks) * inv
        alpha = alpha - alpha.max(-1, keepdims=True)
        ex = jnp.exp(alpha)
        att = ex / ex.sum(-1, keepdims=True)        # [ES, a, b]
        msg = jnp.einsum("eab,ehb->eha", att, Vs)   # [ES, h, a]
        return msg.reshape(-1, H * DV)[None]

    def agg_body(h_l, msg_l, dstl_l, Wo, bo, Wm, bm, g_ln, b_ln,
                 g_mlp, b_mlp):
        h_loc = h_l[0]
        msg_l = msg_l[0]
        dstl_l = dstl_l[0]
        agg = jax.ops.segment_sum(msg_l, dstl_l, num_segments=NLOC + 1,
                                  indices_are_sorted=True)[:NLOC]
        h1 = ln(h_loc + agg @ Wo.T + bo, g_ln, b_ln)
        h2 = h1 + h1 @ Wm.T + bm
        h_loc = ln(h2, g_mlp, b_mlp)
        return h_loc[None]

    jembed = jax.jit(shard_map(
        embed_body, mesh=mesh, in_specs=(P("x"),), out_specs=P("x"),
        check_rep=False,
    ))
    jedge = jax.jit(shard_map(
        edge_body, mesh=mesh,
        in_specs=(P("x"), P("x"), P("x")) + (P(),) * 3,
        out_specs=P("x"),
        check_rep=False,
    ))
    jagg = jax.jit(shard_map(
        agg_body, mesh=mesh,
        in_specs=(P("x"), P("x"), P("x")) + (P(),) * 8,
        out_specs=P("x"),
        check_rep=False,
    ))

    def run_all():
        h = jembed(x_sh)
        for l in range(L):
            msg = jedge(h, src_sh, dstl_sh, Wq[l], Wk[l], Wv[l])
            h = jagg(h, msg, dstl_sh, Wo[l], bo[l], Wm[l], bm[l],
                     g_ln[l], b_ln[l], g_mlp[l], b_mlp[l])
        return h

    out = run_all()
    out.block_until_ready()
    # steady-state timing (first call pays compile + transfers)
    import time
    t0 = time.perf_counter()
    out = run_all()
    out.block_until_ready()
    t1 = time.perf_counter()
    global LAST_HW_EXEC_NS
    LAST_HW_EXEC_NS = int((t1 - t0) * 1e9)
    out = np.asarray(out, np.float32).reshape(N, D)
    if not np.all(np.isfinite(out)):
        raise RuntimeError("non-finite device output")
    return out


def kernel(x, edge_index, W_embed, Wq, Wk, Wv, Wo, bo, Wm, bm, g_ln, b_ln,
           g_mlp, b_mlp):
    x = np.asarray(x, np.float32)
    W_embed = np.asarray(W_embed, np.float32)
    Wq = np.asarray(Wq, np.float32)
    Wk = np.asarray(Wk, np.float32)
    Wv = np.asarray(Wv, np.float32)
    Wo = np.asarray(Wo, np.float32)
    bo = np.asarray(bo, np.float32)
    Wm = np.asarray(Wm, np.float32)
    bm = np.asarray(bm, np.float32)
    g_ln = np.asarray(g_ln, np.float32)
    b_ln = np.asarray(b_ln, np.float32)
    g_mlp = np.asarray(g_mlp, np.float32)
    b_mlp = np.asarray(b_mlp, np.float32)
    ei = np.asarray(edge_index)
    src = ei[0].astype(np.int64)
    dst = ei[1].astype(np.int64)

    if os.environ.get("KERNEL_FORCE_HOST") != "1":
        try:
            return _device_kernel(x, src, dst, W_embed, Wq, Wk, Wv, Wo, bo,
                                  Wm, bm, g_ln, b_ln, g_mlp, b_mlp)
        except Exception as e:
            import traceback
            print("device path failed, falling back to host:", e)
            traceback.print_exc()
    return _host_kernel(x, src, dst, W_embed, Wq, Wk, Wv, Wo, bo, Wm, bm,
                        g_ln, b_ln, g_mlp, b_mlp)

